# revision 1
# baseline (speedup 1.0000x reference)
"""GCN-VAE encoder (2x GCNConv+tanh, then mean/logvar GCNConv heads) on 8
Trainium2 NeuronCores via Bass/Tile.

Strategy:
  - Nodes sharded 6250/core (padded to 6272 = 49*128); small weights replicated.
  - Per pass, propagation out = A_norm @ z is computed per dst-shard:
    edges bucketed by (dst owner core, dst 128-row group), sorted by src.
    Per 128-edge chunk: indirect-DMA gather of z[src] rows (bf16) into a
    [128 edges x F] SBUF tile, then one PE matmul with a host-precomputed
    selection matrix S'[edge, dst_slot] = norm (0 for padding) accumulating
    into the group's PSUM tile.
  - Dense z = h @ W runs sharded on each core (lhsT = PE-transposed h blocks,
    W streams as rhs); the full z needed for the next gather is assembled
    with an AllGather collective across the 8 cores.
  - mean/logvar heads share one propagation over concat(h@Wm, h@Wv) (256 cols).
"""
import sys
import types
import numpy as np
import ml_dtypes
from contextlib import ExitStack

# antenv.axon_hooks shim: run_bass_kernel_spmd(trace=True) under axon needs it;
# harmless if never used (kernel runs trace=False).
try:
    import antenv  # noqa: E402
except ImportError:
    antenv = None
if antenv is not None and "antenv.axon_hooks" not in sys.modules:
    _hooks_mod = types.ModuleType("antenv.axon_hooks")
    _hooks_mod._hook = None

    def _set_hook(h):
        _hooks_mod._hook = h

    def _get_hook():
        if _hooks_mod._hook is None:
            try:
                from trn_agent_boot.trn_boot import _ntff_profile_via_ctypes
                _hooks_mod._hook = _ntff_profile_via_ctypes(
                    "/opt/axon/libaxon_pjrt.so")
            except Exception:
                return None
        return _hooks_mod._hook

    _hooks_mod.set_axon_ntff_profile_hook = _set_hook
    _hooks_mod.get_axon_ntff_profile_hook = _get_hook
    sys.modules["antenv.axon_hooks"] = _hooks_mod
    antenv.axon_hooks = _hooks_mod

import concourse.bass as bass
import concourse.tile as tile
from concourse import bacc, mybir
from concourse.bass_utils import run_bass_kernel_spmd
from concourse.tile_rust import add_dep_helper

P = 128
NC = 8
DH = 512
DZ = 128
FMV = 2 * DZ
KT = DH // P          # 4 k-tiles of the hidden dim
BF16 = mybir.dt.bfloat16
F32 = mybir.dt.float32
I32 = mybir.dt.int32


def _build_program(N, Cg):
    """Build + compile the SPMD Bass program. Cg: chunks per dst group
    (same for every core; per-group)."""
    NS = N // NC                      # owned rows per core
    G = (NS + P - 1) // P             # dst groups per core
    NSP = G * P                       # padded shard rows
    NPAD = NC * NSP                   # padded global rows (AllGather layout)
    G = len(Cg)
    colst = np.concatenate([[0], np.cumsum(Cg)]).astype(int)
    Ctot = int(colst[-1])

    nc = bacc.Bacc("TRN2", target_bir_lowering=False, debug=False,
                   num_devices=NC)

    din = lambda n, s, d: nc.declare_dram_parameter(n, list(s), d, isOutput=False)
    dout = lambda n, s, d: nc.declare_dram_parameter(n, list(s), d, isOutput=True)

    xt = din("xt", [DH, NSP], BF16)
    w1 = din("w1", [DH, DH], BF16)
    w2 = din("w2", [DH, DH], BF16)
    wmv = din("wmv", [DH, FMV], BF16)
    b1b = din("b1b", [P, DH], F32)
    b2b = din("b2b", [P, DH], F32)
    bmvb = din("bmvb", [P, FMV], F32)
    noi = din("noi", [NSP, DZ], F32)
    srcx = din("srcx", [P, Ctot], I32)
    spv = din("spv", [P, Ctot * P], BF16)
    ident = din("ident", [P, P], BF16)
    oz = dout("oz", [NSP, DZ], F32)
    om = dout("om", [NSP, DZ], F32)
    ol = dout("ol", [NSP, DZ], F32)

    z1s = nc.dram_tensor("z1s", [NSP, DH], BF16)
    z1f = nc.dram_tensor("z1f", [NPAD, DH], BF16, addr_space="Shared")
    z2s = nc.dram_tensor("z2s", [NSP, DH], BF16)
    z2f = nc.dram_tensor("z2f", [NPAD, DH], BF16, addr_space="Shared")
    zms = nc.dram_tensor("zms", [NSP, FMV], BF16)
    zmf = nc.dram_tensor("zmf", [NPAD, FMV], BF16, addr_space="Shared")

    rg = [list(range(NC))]

    with tile.TileContext(nc) as tc, ExitStack() as ctx:
        cpool = ctx.enter_context(tc.tile_pool(name="const", bufs=1))
        xtb_p = ctx.enter_context(tc.tile_pool(name="xtb", bufs=3))
        psd_p = ctx.enter_context(tc.tile_pool(name="psd", bufs=2, space="PSUM"))
        ptr_p = ctx.enter_context(tc.tile_pool(name="ptr", bufs=2, space="PSUM"))
        pgp_p = ctx.enter_context(tc.tile_pool(name="pgp", bufs=2, space="PSUM"))
        zsb_p = ctx.enter_context(tc.tile_pool(name="zsb", bufs=3))
        msg_p = ctx.enter_context(tc.tile_pool(name="msg", bufs=12))
        spt_p = ctx.enter_context(tc.tile_pool(name="spt", bufs=2))
        tmp_p = ctx.enter_context(tc.tile_pool(name="tmp", bufs=3))
        htl_p = ctx.enter_context(tc.tile_pool(name="htl", bufs=6))

        # ---- resident constants ----
        w1t = cpool.tile([P, KT * DH], BF16)
        w2t = cpool.tile([P, KT * DH], BF16)
        wmvt = cpool.tile([P, KT * FMV], BF16)
        for k in range(KT):
            nc.sync.dma_start(out=w1t[:, k * DH:(k + 1) * DH],
                              in_=w1[k * P:(k + 1) * P, :])
            nc.sync.dma_start(out=w2t[:, k * DH:(k + 1) * DH],
                              in_=w2[k * P:(k + 1) * P, :])
            nc.sync.dma_start(out=wmvt[:, k * FMV:(k + 1) * FMV],
                              in_=wmv[k * P:(k + 1) * P, :])
        b1t = cpool.tile([P, DH], F32)
        nc.sync.dma_start(out=b1t[:], in_=b1b[:, :])
        b2t = cpool.tile([P, DH], F32)
        nc.sync.dma_start(out=b2t[:], in_=b2b[:, :])
        bmvt = cpool.tile([P, FMV], F32)
        nc.sync.dma_start(out=bmvt[:], in_=bmvb[:, :])
        idt = cpool.tile([P, P], BF16)
        nc.sync.dma_start(out=idt[:], in_=ident[:, :])
        idxt = cpool.tile([P, Ctot], I32)
        nc.sync.dma_start(out=idxt[:], in_=srcx[:, :])

        def dense_from_xt(wt, out_dram, Fo):
            """out = x_shard @ W; lhsT blocks come directly from xt."""
            writes = []
            for m in range(G):
                xtb = xtb_p.tile([P, KT * P], BF16, tag="xtb")
                for k in range(KT):
                    nc.sync.dma_start(
                        out=xtb[:, k * P:(k + 1) * P],
                        in_=xt[k * P:(k + 1) * P, m * P:(m + 1) * P])
                ps = psd_p.tile([P, Fo], F32, tag="psd")
                for k in range(KT):
                    nc.tensor.matmul(out=ps[:],
                                     lhsT=xtb[:, k * P:(k + 1) * P],
                                     rhs=wt[:, k * Fo:(k + 1) * Fo],
                                     start=(k == 0), stop=(k == KT - 1))
                zb = zsb_p.tile([P, Fo], BF16, tag="zsb")
                nc.vector.tensor_copy(out=zb[:], in_=ps[:])
                wr = nc.sync.dma_start(out=out_dram[m * P:(m + 1) * P, :],
                                       in_=zb[:])
                writes.append(wr)
            return writes

        def dense_tile_from_sbuf(hb, wt, out_dram, m, Fo):
            """One dense output tile from an SBUF-resident h tile: PE-transpose
            the 4 k-blocks, accumulate lhsT.T @ W into PSUM, store bf16."""
            ps = psd_p.tile([P, Fo], F32, tag="psd")
            for k in range(KT):
                tp = ptr_p.tile([P, P], BF16, tag="ptr")
                nc.tensor.transpose(out=tp[:],
                                    in_=hb[:, k * P:(k + 1) * P],
                                    identity=idt[:])
                ht = htl_p.tile([P, P], BF16, tag="htl")
                nc.vector.tensor_copy(out=ht[:], in_=tp[:])
                nc.tensor.matmul(out=ps[:], lhsT=ht[:],
                                 rhs=wt[:, k * Fo:(k + 1) * Fo],
                                 start=(k == 0), stop=(k == KT - 1))
            zb = zsb_p.tile([P, Fo], BF16, tag="zsb")
            nc.vector.tensor_copy(out=zb[:], in_=ps[:])
            return nc.sync.dma_start(out=out_dram[m * P:(m + 1) * P, :],
                                     in_=zb[:])

        def all_gather(src_dram, dst_dram, shard_writes):
            cc = nc.gpsimd.collective_compute(
                "AllGather", mybir.AluOpType.bypass, replica_groups=rg,
                ins=[src_dram.ap().opt()], outs=[dst_dram.ap().opt()])
            for wr in shard_writes:
                add_dep_helper(cc.ins, wr.ins, reason="AG after shard writes")
            return cc

        def prop_pass(zf_dram, F, ag, epilogue):
            """out[g] = sum_chunks S'.T @ z[src]; epilogue(g, psum_tile)."""
            for g in range(G):
                c0 = int(colst[g])
                cn = int(Cg[g])
                sp = spt_p.tile([P, cn * P], BF16, tag="spt")
                nc.sync.dma_start(out=sp[:],
                                  in_=spv[:, c0 * P:(c0 + cn) * P])
                ps = pgp_p.tile([P, F], F32, tag="pgp")
                for j in range(cn):
                    jj = c0 + j
                    msg = msg_p.tile([P, F], BF16, tag="msg")
                    gt = nc.gpsimd.indirect_dma_start(
                        out=msg[:], out_offset=None, in_=zf_dram[:, :],
                        in_offset=bass.IndirectOffsetOnAxis(
                            ap=idxt[:, jj:jj + 1], axis=0))
                    if ag is not None:
                        add_dep_helper(gt.ins, ag.ins,
                                       reason="gather after AG")
                    nc.tensor.matmul(out=ps[:],
                                     lhsT=sp[:, j * P:(j + 1) * P],
                                     rhs=msg[:],
                                     start=(j == 0), stop=(j == cn - 1))
                epilogue(g, ps)

        # ---- layer 1 ----
        w1_writes = dense_from_xt(w1t, z1s, DH)
        ag1 = all_gather(z1s, z1f, w1_writes)

        def epi_tanh_dense(bias_t, wt, out_dram, Fo, writes):
            """tanh epilogue fused with the NEXT layer's dense tile so the
            dense work interleaves into the pass (PE issues in program
            order; emitting it here lets it hide under the gather stream)."""
            def _e(g, ps):
                t1 = tmp_p.tile([P, DH], F32, tag="tmp")
                nc.vector.tensor_tensor(out=t1[:], in0=ps[:], in1=bias_t[:],
                                        op=mybir.AluOpType.add)
                hs = zsb_p.tile([P, DH], BF16, tag="hsb")
                nc.scalar.activation(out=hs[:], in_=t1[:],
                                     func=mybir.ActivationFunctionType.Tanh)
                writes[g] = dense_tile_from_sbuf(hs, wt, out_dram, g, Fo)
            return _e

        w2_writes = [None] * G
        prop_pass(z1f, DH, ag1, epi_tanh_dense(b1t, w2t, z2s, DH, w2_writes))
        ag2 = all_gather(z2s, z2f, w2_writes)

        mv_writes = [None] * G
        prop_pass(z2f, DH, ag2, epi_tanh_dense(b2t, wmvt, zms, FMV, mv_writes))
        ag3 = all_gather(zms, zmf, mv_writes)

        def epi_mv(g, ps):
            mean = tmp_p.tile([P, DZ], F32, tag="mean")
            nc.vector.tensor_tensor(out=mean[:], in0=ps[:, :DZ],
                                    in1=bmvt[:, :DZ], op=mybir.AluOpType.add)
            lgv = tmp_p.tile([P, DZ], F32, tag="lgv")
            nc.vector.tensor_tensor(out=lgv[:], in0=ps[:, DZ:],
                                    in1=bmvt[:, DZ:], op=mybir.AluOpType.add)
            ex = tmp_p.tile([P, DZ], F32, tag="ex")
            nc.scalar.activation(out=ex[:], in_=lgv[:],
                                 func=mybir.ActivationFunctionType.Exp,
                                 scale=0.5)
            nt = tmp_p.tile([P, DZ], F32, tag="nt")
            nc.sync.dma_start(out=nt[:], in_=noi[g * P:(g + 1) * P, :])
            zt = tmp_p.tile([P, DZ], F32, tag="zt")
            nc.vector.tensor_tensor(out=zt[:], in0=nt[:], in1=ex[:],
                                    op=mybir.AluOpType.mult)
            nc.vector.tensor_tensor(out=zt[:], in0=zt[:], in1=mean[:],
                                    op=mybir.AluOpType.add)
            nc.sync.dma_start(out=oz[g * P:(g + 1) * P, :], in_=zt[:])
            nc.sync.dma_start(out=om[g * P:(g + 1) * P, :], in_=mean[:])
            nc.sync.dma_start(out=ol[g * P:(g + 1) * P, :], in_=lgv[:])

        prop_pass(zmf, FMV, ag3, epi_mv)

    nc.compile()
    return nc


def _preprocess(N, x, edge_index, noise):
    """Bucket edges by (dst owner, dst group), sorted by src; build per-core
    gather-index + selection-matrix arrays."""
    NS = N // NC
    G = (NS + P - 1) // P
    NSP = G * P
    src = np.concatenate([np.asarray(edge_index[0]),
                          np.arange(N)]).astype(np.int64)
    dst = np.concatenate([np.asarray(edge_index[1]),
                          np.arange(N)]).astype(np.int64)
    deg = np.bincount(dst, minlength=N).astype(np.float32)
    dinv = np.where(deg > 0, 1.0 / np.sqrt(deg), 0.0).astype(np.float32)
    normv = (dinv[src] * dinv[dst]).astype(np.float32)

    owner = dst // NS
    ldst = dst - owner * NS
    grp = ldst >> 7
    dstrel = (ldst & 127).astype(np.int64)
    bucket = owner * G + grp
    order = np.argsort(bucket * np.int64(N) + src, kind="stable")
    src, normv, owner, dstrel, bucket = (
        src[order], normv[order], owner[order], dstrel[order], bucket[order])
    grp = bucket - owner * G

    counts = np.bincount(bucket, minlength=NC * G).reshape(NC, G)
    Cg = np.maximum(1, (counts.max(axis=0) + P - 1) // P).astype(int)
    colst = np.concatenate([[0], np.cumsum(Cg)]).astype(int)
    Ctot = int(colst[-1])

    # rank within bucket for every (sorted) edge
    bstart = np.concatenate([[0], np.cumsum(counts.reshape(-1))])
    rank = np.arange(len(src)) - bstart[bucket]

    srcp = ((src // NS) * NSP + (src - (src // NS) * NS)).astype(np.int32)

    src_arr = np.zeros((NC, P, Ctot), np.int32)
    sp_arr = np.zeros((NC, P, Ctot * P), ml_dtypes.bfloat16)
    col = colst[grp] + (rank >> 7)
    prt = rank & 127
    src_arr[owner, prt, col] = srcp
    sp_arr[owner, prt, col * P + dstrel] = normv

    return NS, G, NSP, Cg, src_arr, sp_arr


_PROGRAM_CACHE = {}
LAST_RESULTS = None


def kernel(x, edge_index, noise, W1, b1, W2, b2, Wm, bm, Wv, bv):
    x = np.asarray(x, np.float32)
    noise = np.asarray(noise, np.float32)
    N = x.shape[0]

    NS, G, NSP, Cg, src_arr, sp_arr = _preprocess(N, x, edge_index, noise)

    key = (N, tuple(Cg))
    if key not in _PROGRAM_CACHE:
        _PROGRAM_CACHE[key] = _build_program(N, Cg)
    nc = _PROGRAM_CACHE[key]

    bf = ml_dtypes.bfloat16
    xt_all = np.ascontiguousarray(x.T).astype(bf)
    w1_ = np.asarray(W1, np.float32).astype(bf)
    w2_ = np.asarray(W2, np.float32).astype(bf)
    wmv_ = np.concatenate([np.asarray(Wm, np.float32),
                           np.asarray(Wv, np.float32)], axis=1).astype(bf)
    b1b = np.ascontiguousarray(
        np.broadcast_to(np.asarray(b1, np.float32), (P, DH)))
    b2b = np.ascontiguousarray(
        np.broadcast_to(np.asarray(b2, np.float32), (P, DH)))
    bmvb = np.ascontiguousarray(np.broadcast_to(
        np.concatenate([np.asarray(bm, np.float32),
                        np.asarray(bv, np.float32)]), (P, FMV)))
    ident = np.eye(P, dtype=bf)

    in_maps = []
    for c in range(NC):
        xts = np.zeros((DH, NSP), bf)
        xts[:, :NS] = xt_all[:, c * NS:(c + 1) * NS]
        nois = np.zeros((NSP, DZ), np.float32)
        nois[:NS] = noise[c * NS:(c + 1) * NS]
        in_maps.append({
            "xt": xts, "w1": w1_, "w2": w2_, "wmv": wmv_,
            "b1b": b1b, "b2b": b2b, "bmvb": bmvb, "noi": nois,
            "srcx": src_arr[c], "spv": sp_arr[c], "ident": ident,
        })

    res = run_bass_kernel_spmd(nc, in_maps, core_ids=list(range(NC)))
    global LAST_RESULTS
    LAST_RESULTS = res

    z = np.empty((N, DZ), np.float32)
    mean = np.empty((N, DZ), np.float32)
    logvar = np.empty((N, DZ), np.float32)
    for c in range(NC):
        z[c * NS:(c + 1) * NS] = res.results[c]["oz"][:NS]
        mean[c * NS:(c + 1) * NS] = res.results[c]["om"][:NS]
        logvar[c * NS:(c + 1) * NS] = res.results[c]["ol"][:NS]
    return (z, mean, logvar)



# revision 11
# speedup vs baseline: 1.2493x; 1.2493x over previous
"""GCN-VAE encoder (2x GCNConv+tanh, then mean/logvar GCNConv heads) on 8
Trainium2 NeuronCores via Bass/Tile.

Strategy:
  - Nodes sharded 6250/core (padded to 6272 = 49*128); small weights replicated.
  - Per pass, propagation out = A_norm @ z is computed per dst-shard:
    edges bucketed by (dst owner core, dst 128-row group), sorted by src.
    Per 128-edge chunk: indirect-DMA gather of z[src] rows (bf16) into a
    [128 edges x F] SBUF tile, then one PE matmul with a host-precomputed
    selection matrix S'[edge, dst_slot] = norm (0 for padding) accumulating
    into the group's PSUM tile.
  - Dense z = h @ W runs sharded on each core (lhsT = PE-transposed h blocks,
    W streams as rhs); the full z needed for the next gather is assembled
    with an AllGather collective across the 8 cores.
  - mean/logvar heads share one propagation over concat(h@Wm, h@Wv) (256 cols).
"""
import sys
import types
import numpy as np
import ml_dtypes
from contextlib import ExitStack

# antenv.axon_hooks shim: run_bass_kernel_spmd(trace=True) under axon needs it;
# harmless if never used (kernel runs trace=False).
try:
    import antenv  # noqa: E402
except ImportError:
    antenv = None
if antenv is not None and "antenv.axon_hooks" not in sys.modules:
    _hooks_mod = types.ModuleType("antenv.axon_hooks")
    _hooks_mod._hook = None

    def _set_hook(h):
        _hooks_mod._hook = h

    def _get_hook():
        if _hooks_mod._hook is None:
            try:
                from trn_agent_boot.trn_boot import _ntff_profile_via_ctypes
                _hooks_mod._hook = _ntff_profile_via_ctypes(
                    "/opt/axon/libaxon_pjrt.so")
            except Exception:
                return None
        return _hooks_mod._hook

    _hooks_mod.set_axon_ntff_profile_hook = _set_hook
    _hooks_mod.get_axon_ntff_profile_hook = _get_hook
    sys.modules["antenv.axon_hooks"] = _hooks_mod
    antenv.axon_hooks = _hooks_mod

import concourse.bass as bass
import concourse.tile as tile
from concourse import bacc, mybir
from concourse.bass_utils import run_bass_kernel_spmd
from concourse.tile_rust import add_dep_helper

P = 128
NC = 8
DH = 512
DZ = 128
FMV = 2 * DZ
KT = DH // P          # 4 k-tiles of the hidden dim
BF16 = mybir.dt.bfloat16
F32 = mybir.dt.float32
I32 = mybir.dt.int32


IXSPLIT = 32768   # int16 dma_gather index limit: rows >= this use a
                  # row-sliced src AP with rebased indices


def _build_program(N, CgL, CgH):
    """Build + compile the SPMD Bass program. CgL/CgH: lo/hi chunk-columns
    per dst group (same for every core; per-group)."""
    NS = N // NC                      # owned rows per core
    G = (NS + P - 1) // P             # dst groups per core
    NSP = G * P                       # padded shard rows
    NPAD = NC * NSP                   # padded global rows (AllGather layout)
    Cg = [int(a) + int(b) for a, b in zip(CgL, CgH)]
    G = len(Cg)
    colst = np.concatenate([[0], np.cumsum(Cg)]).astype(int)
    Ctot = int(colst[-1])

    nc = bacc.Bacc("TRN2", target_bir_lowering=False, debug=False,
                   num_devices=NC)

    din = lambda n, s, d: nc.declare_dram_parameter(n, list(s), d, isOutput=False)
    dout = lambda n, s, d: nc.declare_dram_parameter(n, list(s), d, isOutput=True)

    xt = din("xt", [DH, NSP], BF16)
    w1 = din("w1", [DH, DH], BF16)
    w2 = din("w2", [DH, DH], BF16)
    wmv = din("wmv", [DH, FMV], BF16)
    b1b = din("b1b", [P, DH], F32)
    b2b = din("b2b", [P, DH], F32)
    bmvb = din("bmvb", [P, FMV], F32)
    noi = din("noi", [NSP, DZ], F32)
    gix = din("gix", [P, Ctot * 8], mybir.dt.int16)
    spv = din("spv", [P, Ctot * P], BF16)
    ident = din("ident", [P, P], BF16)
    oz = dout("oz", [NSP, DZ], F32)
    om = dout("om", [NSP, DZ], F32)
    ol = dout("ol", [NSP, DZ], F32)

    z1s = nc.dram_tensor("z1s", [NSP, DH], BF16)
    z1f = nc.dram_tensor("z1f", [NPAD, DH], BF16, addr_space="Shared")
    z2s = nc.dram_tensor("z2s", [NSP, DH], BF16)
    z2f = nc.dram_tensor("z2f", [NPAD, DH], BF16, addr_space="Shared")
    zms = nc.dram_tensor("zms", [NSP, FMV], BF16)
    zmf = nc.dram_tensor("zmf", [NPAD, FMV], BF16, addr_space="Shared")

    rg = [list(range(NC))]

    with tile.TileContext(nc) as tc, ExitStack() as ctx:
        cpool = ctx.enter_context(tc.tile_pool(name="const", bufs=1))
        xtb_p = ctx.enter_context(tc.tile_pool(name="xtb", bufs=3))
        psd_p = ctx.enter_context(tc.tile_pool(name="psd", bufs=2, space="PSUM"))
        ptr_p = ctx.enter_context(tc.tile_pool(name="ptr", bufs=2, space="PSUM"))
        pgp_p = ctx.enter_context(tc.tile_pool(name="pgp", bufs=2, space="PSUM"))
        zsb_p = ctx.enter_context(tc.tile_pool(name="zsb", bufs=3))
        msg_p = ctx.enter_context(tc.tile_pool(name="msg", bufs=2))
        spt_p = ctx.enter_context(tc.tile_pool(name="spt", bufs=2))
        tmp_p = ctx.enter_context(tc.tile_pool(name="tmp", bufs=3))
        htl_p = ctx.enter_context(tc.tile_pool(name="htl", bufs=6))

        # ---- resident constants ----
        w1t = cpool.tile([P, KT * DH], BF16)
        w2t = cpool.tile([P, KT * DH], BF16)
        wmvt = cpool.tile([P, KT * FMV], BF16)
        for k in range(KT):
            nc.sync.dma_start(out=w1t[:, k * DH:(k + 1) * DH],
                              in_=w1[k * P:(k + 1) * P, :])
            nc.sync.dma_start(out=w2t[:, k * DH:(k + 1) * DH],
                              in_=w2[k * P:(k + 1) * P, :])
            nc.sync.dma_start(out=wmvt[:, k * FMV:(k + 1) * FMV],
                              in_=wmv[k * P:(k + 1) * P, :])
        b1t = cpool.tile([P, DH], F32)
        nc.sync.dma_start(out=b1t[:], in_=b1b[:, :])
        b2t = cpool.tile([P, DH], F32)
        nc.sync.dma_start(out=b2t[:], in_=b2b[:, :])
        bmvt = cpool.tile([P, FMV], F32)
        nc.sync.dma_start(out=bmvt[:], in_=bmvb[:, :])
        idt = cpool.tile([P, P], BF16)
        nc.sync.dma_start(out=idt[:], in_=ident[:, :])
        gixt = cpool.tile([P, Ctot * 8], mybir.dt.int16)
        nc.sync.dma_start(out=gixt[:], in_=gix[:, :])

        def dense_from_xt(wt, out_dram, Fo):
            """out = x_shard @ W; lhsT blocks come directly from xt."""
            writes = []
            for m in range(G):
                xtb = xtb_p.tile([P, KT * P], BF16, tag="xtb")
                for k in range(KT):
                    nc.sync.dma_start(
                        out=xtb[:, k * P:(k + 1) * P],
                        in_=xt[k * P:(k + 1) * P, m * P:(m + 1) * P])
                ps = psd_p.tile([P, Fo], F32, tag="psd")
                for k in range(KT):
                    nc.tensor.matmul(out=ps[:],
                                     lhsT=xtb[:, k * P:(k + 1) * P],
                                     rhs=wt[:, k * Fo:(k + 1) * Fo],
                                     start=(k == 0), stop=(k == KT - 1))
                zb = zsb_p.tile([P, Fo], BF16, tag="zsb")
                nc.vector.tensor_copy(out=zb[:], in_=ps[:])
                wr = nc.sync.dma_start(out=out_dram[m * P:(m + 1) * P, :],
                                       in_=zb[:])
                writes.append(wr)
            return writes

        def dense_tile_from_sbuf(hb, wt, out_dram, m, Fo):
            """One dense output tile from an SBUF-resident h tile: PE-transpose
            the 4 k-blocks, accumulate lhsT.T @ W into PSUM, store bf16."""
            ps = psd_p.tile([P, Fo], F32, tag="psd")
            for k in range(KT):
                tp = ptr_p.tile([P, P], BF16, tag="ptr")
                nc.tensor.transpose(out=tp[:],
                                    in_=hb[:, k * P:(k + 1) * P],
                                    identity=idt[:])
                ht = htl_p.tile([P, P], BF16, tag="htl")
                nc.vector.tensor_copy(out=ht[:], in_=tp[:])
                nc.tensor.matmul(out=ps[:], lhsT=ht[:],
                                 rhs=wt[:, k * Fo:(k + 1) * Fo],
                                 start=(k == 0), stop=(k == KT - 1))
            zb = zsb_p.tile([P, Fo], BF16, tag="zsb")
            nc.vector.tensor_copy(out=zb[:], in_=ps[:])
            return nc.sync.dma_start(out=out_dram[m * P:(m + 1) * P, :],
                                     in_=zb[:])

        def all_gather(src_dram, dst_dram, shard_writes):
            cc = nc.gpsimd.collective_compute(
                "AllGather", mybir.AluOpType.bypass, replica_groups=rg,
                ins=[src_dram.ap().opt()], outs=[dst_dram.ap().opt()])
            for wr in shard_writes:
                add_dep_helper(cc.ins, wr.ins, reason="AG after shard writes")
            return cc

        def prop_pass(zf_dram, F, ag, epilogue):
            """out[g] = sum_chunks S'.T @ z[src]; epilogue(g, psum_tile).

            Per dst group, two dma_gather ucode instructions (lo rows
            < IXSPLIT, hi rows rebased against a sliced src AP) fetch all
            cn*128 message rows. dma_gather packs ~16 rows/descriptor, so
            a whole group costs ~1us of GpSimd vs ~1us per 128 rows with
            indirect_dma_start."""
            for g in range(G):
                c0 = int(colst[g])
                cl = int(CgL[g])
                ch = int(CgH[g])
                cn = cl + ch
                sp = spt_p.tile([P, cn * P], BF16, tag="spt")
                nc.sync.dma_start(out=sp[:],
                                  in_=spv[:, c0 * P:(c0 + cn) * P])
                msg = msg_p.tile([P, cn, F], BF16, tag="msg")
                # single_packet=True flakily hangs at large num_idxs
                g1 = nc.gpsimd.dma_gather(
                    out_ap=msg[:, 0:cl, :], in_ap=zf_dram[:, :],
                    idxs_ap=gixt[:, 8 * c0:8 * (c0 + cl)],
                    num_idxs=cl * P, num_idxs_reg=cl * P, elem_size=F,
                    single_packet=False)
                g2 = nc.gpsimd.dma_gather(
                    out_ap=msg[:, cl:cn, :], in_ap=zf_dram[IXSPLIT:, :],
                    idxs_ap=gixt[:, 8 * (c0 + cl):8 * (c0 + cn)],
                    num_idxs=ch * P, num_idxs_reg=ch * P, elem_size=F,
                    single_packet=False)
                if ag is not None:
                    add_dep_helper(g1.ins, ag.ins, reason="gather after AG")
                    add_dep_helper(g2.ins, ag.ins, reason="gather after AG")
                ps = pgp_p.tile([P, F], F32, tag="pgp")
                for j in range(cn):
                    nc.tensor.matmul(out=ps[:],
                                     lhsT=sp[:, j * P:(j + 1) * P],
                                     rhs=msg[:, j, :],
                                     start=(j == 0), stop=(j == cn - 1))
                epilogue(g, ps)

        # ---- layer 1 ----
        w1_writes = dense_from_xt(w1t, z1s, DH)
        ag1 = all_gather(z1s, z1f, w1_writes)

        def epi_tanh_dense(bias_t, wt, out_dram, Fo, writes):
            """tanh epilogue fused with the NEXT layer's dense tile so the
            dense work interleaves into the pass (PE issues in program
            order; emitting it here lets it hide under the gather stream)."""
            def _e(g, ps):
                t1 = tmp_p.tile([P, DH], F32, tag="tmp")
                nc.vector.tensor_tensor(out=t1[:], in0=ps[:], in1=bias_t[:],
                                        op=mybir.AluOpType.add)
                hs = zsb_p.tile([P, DH], BF16, tag="hsb")
                nc.scalar.activation(out=hs[:], in_=t1[:],
                                     func=mybir.ActivationFunctionType.Tanh)
                writes[g] = dense_tile_from_sbuf(hs, wt, out_dram, g, Fo)
            return _e

        w2_writes = [None] * G
        prop_pass(z1f, DH, ag1, epi_tanh_dense(b1t, w2t, z2s, DH, w2_writes))
        ag2 = all_gather(z2s, z2f, w2_writes)

        mv_writes = [None] * G
        prop_pass(z2f, DH, ag2, epi_tanh_dense(b2t, wmvt, zms, FMV, mv_writes))
        ag3 = all_gather(zms, zmf, mv_writes)

        def epi_mv(g, ps):
            mean = tmp_p.tile([P, DZ], F32, tag="mean")
            nc.vector.tensor_tensor(out=mean[:], in0=ps[:, :DZ],
                                    in1=bmvt[:, :DZ], op=mybir.AluOpType.add)
            lgv = tmp_p.tile([P, DZ], F32, tag="lgv")
            nc.vector.tensor_tensor(out=lgv[:], in0=ps[:, DZ:],
                                    in1=bmvt[:, DZ:], op=mybir.AluOpType.add)
            ex = tmp_p.tile([P, DZ], F32, tag="ex")
            nc.scalar.activation(out=ex[:], in_=lgv[:],
                                 func=mybir.ActivationFunctionType.Exp,
                                 scale=0.5)
            nt = tmp_p.tile([P, DZ], F32, tag="nt")
            nc.sync.dma_start(out=nt[:], in_=noi[g * P:(g + 1) * P, :])
            zt = tmp_p.tile([P, DZ], F32, tag="zt")
            nc.vector.tensor_tensor(out=zt[:], in0=nt[:], in1=ex[:],
                                    op=mybir.AluOpType.mult)
            nc.vector.tensor_tensor(out=zt[:], in0=zt[:], in1=mean[:],
                                    op=mybir.AluOpType.add)
            nc.sync.dma_start(out=oz[g * P:(g + 1) * P, :], in_=zt[:])
            nc.sync.dma_start(out=om[g * P:(g + 1) * P, :], in_=mean[:])
            nc.sync.dma_start(out=ol[g * P:(g + 1) * P, :], in_=lgv[:])

        prop_pass(zmf, FMV, ag3, epi_mv)

    nc.compile()
    return nc


def _preprocess(N, x, edge_index, noise):
    """Bucket edges by (dst owner, dst group), sorted by src; build per-core
    dma_gather int16 index tables + selection-matrix arrays. Each bucket is
    split into a lo part (gather row < IXSPLIT) and a hi part (row index
    rebased by IXSPLIT), each padded to whole 128-edge chunk columns."""
    NS = N // NC
    G = (NS + P - 1) // P
    NSP = G * P
    src = np.concatenate([np.asarray(edge_index[0]),
                          np.arange(N)]).astype(np.int64)
    dst = np.concatenate([np.asarray(edge_index[1]),
                          np.arange(N)]).astype(np.int64)
    deg = np.bincount(dst, minlength=N).astype(np.float32)
    dinv = np.where(deg > 0, 1.0 / np.sqrt(deg), 0.0).astype(np.float32)
    normv = (dinv[src] * dinv[dst]).astype(np.float32)

    owner = dst // NS
    ldst = dst - owner * NS
    grp = ldst >> 7
    dstrel = (ldst & 127).astype(np.int64)
    bucket = owner * G + grp
    # sort by (bucket, src); srcp is monotone in src so lo rows (srcp <
    # IXSPLIT) form a prefix of each bucket
    order = np.argsort(bucket * np.int64(N) + src, kind="stable")
    src, normv, owner, dstrel, bucket = (
        src[order], normv[order], owner[order], dstrel[order], bucket[order])
    grp = bucket - owner * G

    srcp = ((src // NS) * NSP + (src - (src // NS) * NS)).astype(np.int64)
    islo = srcp < IXSPLIT

    counts = np.bincount(bucket, minlength=NC * G).reshape(NC, G)
    nlo = np.bincount(bucket[islo], minlength=NC * G).reshape(NC, G)
    nhi = counts - nlo
    CgL = np.maximum(1, (nlo.max(axis=0) + P - 1) // P).astype(int)
    CgH = np.maximum(1, (nhi.max(axis=0) + P - 1) // P).astype(int)
    Cg = CgL + CgH
    colst = np.concatenate([[0], np.cumsum(Cg)]).astype(int)
    Ctot = int(colst[-1])

    # rank within bucket for every (sorted) edge
    bstart = np.concatenate([[0], np.cumsum(counts.reshape(-1))])
    rank = np.arange(len(src)) - bstart[bucket]
    rank_hi = rank - nlo.reshape(-1)[bucket]     # valid where ~islo

    # chunk column (shared layout with sp) and partition slot per edge
    lrank = np.where(islo, rank, rank_hi)
    col = colst[grp] + np.where(islo, 0, CgL[grp]) + (lrank >> 7)
    prt = lrank & 127
    ixval = np.where(islo, srcp, srcp - IXSPLIT).astype(np.int16)

    # dma_gather index wrap: flat edge i of an instruction lives at
    # [i % 16, i // 16]; i = local_col*128 + prt, so partition prt % 16,
    # column 8*col + prt // 16. Replicate the 16-row block to 128.
    gidx16 = np.zeros((NC, 16, Ctot * 8), np.int16)
    gidx16[owner, prt % 16, col * 8 + (prt >> 4)] = ixval
    gidx_arr = np.ascontiguousarray(np.tile(gidx16, (1, 8, 1)))

    sp_arr = np.zeros((NC, P, Ctot * P), ml_dtypes.bfloat16)
    sp_arr[owner, prt, col * P + dstrel] = normv

    return NS, G, NSP, CgL, CgH, gidx_arr, sp_arr


_PROGRAM_CACHE = {}
LAST_RESULTS = None


def kernel(x, edge_index, noise, W1, b1, W2, b2, Wm, bm, Wv, bv):
    x = np.asarray(x, np.float32)
    noise = np.asarray(noise, np.float32)
    N = x.shape[0]

    NS, G, NSP, CgL, CgH, gidx_arr, sp_arr = _preprocess(N, x, edge_index,
                                                         noise)

    key = (N, tuple(CgL), tuple(CgH))
    if key not in _PROGRAM_CACHE:
        _PROGRAM_CACHE[key] = _build_program(N, CgL, CgH)
    nc = _PROGRAM_CACHE[key]

    bf = ml_dtypes.bfloat16
    xt_all = np.ascontiguousarray(x.T).astype(bf)
    w1_ = np.asarray(W1, np.float32).astype(bf)
    w2_ = np.asarray(W2, np.float32).astype(bf)
    wmv_ = np.concatenate([np.asarray(Wm, np.float32),
                           np.asarray(Wv, np.float32)], axis=1).astype(bf)
    b1b = np.ascontiguousarray(
        np.broadcast_to(np.asarray(b1, np.float32), (P, DH)))
    b2b = np.ascontiguousarray(
        np.broadcast_to(np.asarray(b2, np.float32), (P, DH)))
    bmvb = np.ascontiguousarray(np.broadcast_to(
        np.concatenate([np.asarray(bm, np.float32),
                        np.asarray(bv, np.float32)]), (P, FMV)))
    ident = np.eye(P, dtype=bf)

    in_maps = []
    for c in range(NC):
        xts = np.zeros((DH, NSP), bf)
        xts[:, :NS] = xt_all[:, c * NS:(c + 1) * NS]
        nois = np.zeros((NSP, DZ), np.float32)
        nois[:NS] = noise[c * NS:(c + 1) * NS]
        in_maps.append({
            "xt": xts, "w1": w1_, "w2": w2_, "wmv": wmv_,
            "b1b": b1b, "b2b": b2b, "bmvb": bmvb, "noi": nois,
            "gix": gidx_arr[c], "spv": sp_arr[c], "ident": ident,
        })

    res = run_bass_kernel_spmd(nc, in_maps, core_ids=list(range(NC)))
    global LAST_RESULTS
    LAST_RESULTS = res

    z = np.empty((N, DZ), np.float32)
    mean = np.empty((N, DZ), np.float32)
    logvar = np.empty((N, DZ), np.float32)
    for c in range(NC):
        z[c * NS:(c + 1) * NS] = res.results[c]["oz"][:NS]
        mean[c * NS:(c + 1) * NS] = res.results[c]["om"][:NS]
        logvar[c * NS:(c + 1) * NS] = res.results[c]["ol"][:NS]
    return (z, mean, logvar)

